# revision 19
# baseline (speedup 1.0000x reference)
"""Trainium2 Bass kernel for nn_CumulativeShadeRegressor.

Model (per sample): per-leaf MLP encoder [L, FD] -> [L, H2] (two gelu
layers), softplus absorb/atten heads, a top-to-bottom exponential
transmittance scan over L, mean-pooling over L, and a small dense head on
[Xg | pooled].

Strategy: data-parallel over B across 8 NeuronCores (32 samples/core).
Both the PE and the ACT engine are near-saturated in this problem, so the
kernel attacks both:
  * layer 1 (K=64) runs as row-tiled bf16 matmul pairs (2 concurrent MMs
    in disjoint 64-row PE strips);
  * layer 2 runs in fp8e4 DoubleRow mode (2 MACs/cell, contraction 256
    per pass) with h1 quantized to fp8 by the gelu ACT itself;
  * gelu ACT ops are batched to N=2048 (4 samples per instruction, PSUM
    pair ring of 2x4 banks) to amortize the ~222-cycle ACT init;
  * per-sample pooling is a DVE 3D-AP reduce (off the ACT critical path);
  * absorb/atten head matmuls (M=2) run post-loop as col-tiled bursts
    (4 samples concurrently in 32-col PE strips);
  * the tail uses softplus(x)=ln(1+exp(x)) and T=exp(-cumsum), all inside
    the single natural_log_exp ACT table set (one table switch total).
"""
import sys

sys.path.insert(0, "/opt/trn_rl_repo")

import numpy as np
import ml_dtypes

import concourse.bacc as bacc
import concourse.mybir as mybir
import concourse.tile as tile
from concourse.bass_utils import run_bass_kernel_spmd
from concourse.tile import add_dep_helper

B, L, FD, G = 256, 512, 64, 32
H1, H2, DH = 512, 512, 256
NCORES = 8
BL = B // NCORES          # 32 samples per core
NBLK = BL // 4            # 8 blocks of 4 samples

f32 = mybir.dt.float32
bf16 = mybir.dt.bfloat16
f8e4 = mybir.dt.float8e4
AF = mybir.ActivationFunctionType
ALU = mybir.AluOpType
AX = mybir.AxisListType
DR = mybir.MatmulPerfMode.DoubleRow


def _build():
    nc = bacc.Bacc("TRN2", target_bir_lowering=False, debug=False,
                   num_devices=NCORES)

    d = {}
    d["xlt"] = nc.dram_tensor("xlt", [128, NBLK * 1024], bf16, kind="ExternalInput").ap()
    d["xgt"] = nc.dram_tensor("xgt", [G, BL], f32, kind="ExternalInput").ap()
    d["w1s"] = nc.dram_tensor("w1s", [128, H1], bf16, kind="ExternalInput").ap()
    d["w2f8"] = nc.dram_tensor("w2f8", [128, 2048], f8e4, kind="ExternalInput").ap()
    d["wawt"] = nc.dram_tensor("wawt", [128, 8], bf16, kind="ExternalInput").ap()
    d["wd1g"] = nc.dram_tensor("wd1g", [G, DH], f32, kind="ExternalInput").ap()
    d["wd1p"] = nc.dram_tensor("wd1p", [128, 4 * DH], f32, kind="ExternalInput").ap()
    d["wd2"] = nc.dram_tensor("wd2", [128, 2], f32, kind="ExternalInput").ap()
    d["b1"] = nc.dram_tensor("b1", [128, 4], f32, kind="ExternalInput").ap()
    d["b2"] = nc.dram_tensor("b2", [128, 4], f32, kind="ExternalInput").ap()
    d["bd1"] = nc.dram_tensor("bd1", [128, 2], f32, kind="ExternalInput").ap()
    d["scal"] = nc.dram_tensor("scal", [128, 4], f32, kind="ExternalInput").ap()
    out_d = nc.dram_tensor("out", [BL, 1], f32, kind="ExternalOutput").ap()

    with tile.TileContext(nc) as tc:
        with (
            tc.tile_pool(name="wp", bufs=1) as wp,
            tc.tile_pool(name="pp", bufs=1) as pp,
            tc.tile_pool(name="xp", bufs=4) as xp,
            tc.tile_pool(name="h1p", bufs=2) as h1p,
            tc.tile_pool(name="p2sb", bufs=1) as p2sb,
            tc.tile_pool(name="awp", bufs=2) as awp,
            tc.tile_pool(name="psp", bufs=1, space="PSUM") as psp,
        ):
            w1s_t = wp.tile([128, H1], bf16)
            w2f8_t = wp.tile([128, 2048], f8e4)
            wawt_t = wp.tile([128, 8], bf16)
            xgt_t = wp.tile([G, BL], f32)
            wd1g_t = wp.tile([G, DH], f32)
            wd1p_t = wp.tile([128, 4 * DH], f32)
            wd2_t = wp.tile([128, 2], f32)
            b1_t = wp.tile([128, 4], f32)
            b2_t = wp.tile([128, 4], f32)
            bd1_t = wp.tile([128, 2], f32)
            scal_t = wp.tile([128, 4], f32)

            # scratch + dummy gelu first: the ~1.3us gelu ACT_TABLE_LOAD
            # runs concurrently with the input DMAs
            wu_sb = wp.tile([128, 128], f32, name="wu_sb")
            scr_t = wp.tile([1, 4], f32, name="scr_t")
            texpx = p2sb.tile([BL, L], f32, name="texpx")
            nc.gpsimd.memset(wu_sb[:], 0.0)
            nc.gpsimd.memset(texpx[:, L - 1:L], 1.0)
            nc.scalar.activation(scr_t[0:1, 0:1], wu_sb[0:1, 0:1], AF.Gelu)

            # first block's inputs land before everything else
            x2_pre = {}

            def fetch_x2(g):
                # split across 4 DMA queues so one queue's descriptor
                # stream isn't the critical path
                xt = xp.tile([128, 2 * L], bf16, name=f"x2_{g}", tag="x2")
                for a in range(4):
                    nc.sync.dma_start(
                        xt[32 * a:32 * a + 32, :],
                        d["xlt"][32 * a:32 * a + 32, g * 1024:(g + 1) * 1024])
                x2_pre[g] = xt

            fetch_x2(0)
            for a in range(4):
                nc.sync.dma_start(w1s_t[32 * a:32 * a + 32, :],
                                  d["w1s"][32 * a:32 * a + 32, :])
            nc.sync.dma_start(b1_t[:], d["b1"][:])
            fetch_x2(1)
            fetch_x2(2)
            for nm, t in [("w2f8", w2f8_t), ("b2", b2_t), ("wawt", wawt_t)]:
                nc.sync.dma_start(t[:], d[nm][:])
            for nm, t in [("xgt", xgt_t), ("wd1g", wd1g_t), ("wd1p", wd1p_t),
                          ("wd2", wd2_t), ("bd1", bd1_t), ("scal", scal_t)]:
                nc.gpsimd.dma_start(t[:], d[nm][:])

            pooled_t = pp.tile([128, 4 * BL], f32)   # [h_part, mc*32 + s]
            h2all = pp.tile([128, NBLK * 4 * 2048], bf16)  # [feat, (g*4+mc)*2048 + j*512 + l]

            # all of PSUM as one tile; pairs P0=[0:2048], P1=[2048:4096]
            ps_all = psp.tile([128, 4096], f32)

            # PE warm-up: back-to-back matmuls on scratch data so the HAM
            # clock gate reaches K=8/8 before (and until) the real work.
            for i in range(10):
                nc.tensor.matmul(ps_all[:, 3968:4096], wu_sb[:], wu_sb[:],
                                 start=True, stop=True)

            h1tiles = {}
            unit = 0  # ACT-unit counter; parity picks the PSUM pair

            def l1_unit(g, mc):
                """One layer-1 ACT unit: 4 row-tiled bf16 MM pairs + gelu->fp8."""
                nonlocal unit
                x2t = x2_pre[g]
                h1t = h1tiles[g]
                pbase = (unit % 2) * 2048
                for h in range(2):
                    for sl in range(2):
                        nc.tensor.matmul(
                            ps_all[:, pbase + (2 * h + sl) * 512:
                                   pbase + (2 * h + sl) * 512 + 512],
                            w1s_t[64 * sl:64 * sl + 64, mc * 128:(mc + 1) * 128],
                            x2t[64 * sl:64 * sl + 64, h * 512:(h + 1) * 512],
                            start=True, stop=True)
                nc.scalar.activation(
                    h1t[:, mc * 2048:(mc + 1) * 2048],
                    ps_all[:, pbase:pbase + 2048],
                    AF.Gelu, bias=b1_t[:, mc:mc + 1])
                unit += 1

            def l2_unit(g, mc):
                """One layer-2 ACT unit: 8 fp8 DoubleRow MMs + gelu + pooling."""
                nonlocal unit
                h1t = h1tiles[g]
                pbase = (unit % 2) * 2048
                for kcp in range(2):
                    wk = w2f8_t[:, kcp * 1024:(kcp + 1) * 1024].rearrange(
                        "p (ko mcm) -> p ko mcm", ko=2)
                    hk = h1t[:, (2 * kcp) * 2048:(2 * kcp + 2) * 2048].rearrange(
                        "p (ko n) -> p ko n", ko=2)
                    for j in range(4):
                        nc.tensor.matmul(
                            ps_all[:, pbase + j * 512:pbase + (j + 1) * 512],
                            wk[:, :, mc * 128:(mc + 1) * 128],
                            hk[:, :, j * 512:(j + 1) * 512],
                            start=(kcp == 0), stop=(kcp == 1),
                            perf_mode=DR)
                h2base = (g * 4 + mc) * 2048
                nc.scalar.activation(
                    h2all[:, h2base:h2base + 2048],
                    ps_all[:, pbase:pbase + 2048],
                    AF.Gelu, bias=b2_t[:, mc:mc + 1])
                unit += 1
                # per-sample pooling: sum over L on the DVE
                nc.vector.reduce_sum(
                    pooled_t[:, mc * BL + g * 4:mc * BL + g * 4 + 4],
                    h2all[:, h2base:h2base + 2048].rearrange(
                        "p (j n) -> p j n", j=4),
                    axis=AX.X)

            # prologue: block 0's layer 1, with fillers to bridge the first
            # ACT-latency gap so HAM stays at K=8/8
            h1tiles[0] = h1p.tile([128, 4 * 2048], f8e4, name="h1t_0", tag="h1t")
            for mc in range(4):
                l1_unit(0, mc)
                if mc == 0:
                    for i in range(12):
                        nc.tensor.matmul(ps_all[:, 3968:4096], wu_sb[:],
                                         wu_sb[:], start=True, stop=True)

            # main loop, software-pipelined: block g+1's layer-1 units
            # interleave with block g's layer-2 units so the PE never idles
            # long enough for HAM to re-throttle.
            for g in range(NBLK):
                if g + 3 < NBLK:
                    fetch_x2(g + 3)
                if g + 1 < NBLK:
                    h1tiles[g + 1] = h1p.tile([128, 4 * 2048], f8e4,
                                              name=f"h1t_{g+1}", tag="h1t")
                for mc in range(4):
                    if g + 1 < NBLK:
                        l1_unit(g + 1, mc)
                    l2_unit(g, mc)

            # ---- phase 2 ----
            # absorb/atten pre-acts first (they gate the serial tail; the
            # dense head would head-of-line-block the PE FIFO waiting on
            # pooled_t): col-tiled bursts, 4 samples concurrent.
            # psum bank alternates between [0:512] and [512:1024].
            aw_all = p2sb.tile([64, L], f32)  # rows 0-31 absorb, 32-63 atten
            for g in range(NBLK):
                abank = (g % 2) * 512
                for c in range(4):
                    for j in range(4):
                        nc.tensor.matmul(
                            ps_all[32 * j:32 * j + 2, abank:abank + 512],
                            wawt_t[:, 2 * c:2 * c + 2],
                            h2all[:, (g * 4 + c) * 2048 + j * 512:
                                  (g * 4 + c) * 2048 + (j + 1) * 512],
                            start=(c == 0), stop=(c == 3),
                            tile_position=(0, 32 * j))
                aw_sb = awp.tile([128, L], f32, name=f"aw_sb_{g}", tag="aw_sb")
                # scalar engine (idle here) does the drain; the DVE is still
                # busy with the last block's pooling reduces
                nc.scalar.copy(aw_sb[:], ps_all[:, abank:abank + 512])
                nc.sync.dma_start(aw_all[g * 4:g * 4 + 4, :], aw_sb[0:128:32, :])
                nc.sync.dma_start(aw_all[32 + g * 4:32 + g * 4 + 4, :],
                                  aw_sb[1:128:32, :])

            # dense head (gelu table still resident)
            d1t = []
            gelu_insts = []
            for mc2 in range(2):
                dbase = 1024 + mc2 * 512
                ps = ps_all[:, dbase:dbase + BL]
                nc.tensor.matmul(ps, wd1g_t[:, mc2 * 128:(mc2 + 1) * 128],
                                 xgt_t[:], start=True, stop=False)
                for hc in range(4):
                    nc.tensor.matmul(
                        ps,
                        wd1p_t[:, hc * DH + mc2 * 128:hc * DH + (mc2 + 1) * 128],
                        pooled_t[:, hc * BL:(hc + 1) * BL],
                        start=False, stop=(hc == 3))
                t = p2sb.tile([128, BL], f32, name=f"d1t_{mc2}")
                gi = nc.scalar.activation(t[:], ps, AF.Gelu,
                                          bias=bd1_t[:, mc2:mc2 + 1])
                gelu_insts.append(gi)
                d1t.append(t)
            dps = ps_all[0:BL, 2048:2049]
            nc.tensor.matmul(dps, d1t[0][:], wd2_t[:, 0:1], start=True, stop=False)
            nc.tensor.matmul(dps, d1t[1][:], wd2_t[:, 1:2], start=False, stop=True)

            # tail: softplus via ln(1+exp), T via exp(-cumsum); one table set.
            # Explicitly load the set that serves BOTH Exp and Ln (the
            # automatic chooser would ping-pong between exp-only and
            # ln-only sets, 1.28us per switch).
            # scal col0 rows 0-31 = ba, rows 32-63 = bt.
            from concourse.hw_specs import get_activation_tables
            tabs = get_activation_tables(nc.m.arch)
            set_id = next(i for i, fns in enumerate(tabs.values())
                          if AF.Exp in fns and AF.Ln in fns)
            li = nc.scalar.add_instruction(mybir.InstLoadActFuncSet(
                name=nc.scalar.bass.get_next_instruction_name(),
                act_func_set_id=set_id, ins=[], outs=[]))
            for gi in gelu_insts:
                add_dep_helper(li.ins, gi.ins, sync=True,
                               reason="ACT table set order: gelu before ln/exp")
            e_aw = p2sb.tile([64, L], f32)
            ei = nc.scalar.activation(e_aw[:], aw_all[:], AF.Exp,
                                      bias=scal_t[0:64, 0:1])
            add_dep_helper(ei.ins, li.ins, sync=True,
                           reason="ACT table set order: ln set before exp")
            # softplus = ln(1 + e): the +1 rides the ACT bias for free
            sp_all = p2sb.tile([64, L], f32)
            nc.scalar.activation(sp_all[:], e_aw[:], AF.Ln, bias=1.0)
            # incl[l] = sum_{l'>=l} softplus(atten): add-scan over reversed L
            incl = p2sb.tile([BL, L], f32)
            spt_rev = sp_all[32:64, L - 1::-1]
            incl_rev = incl[:, L - 1::-1]
            nc.vector.tensor_tensor_scan(incl_rev, spt_rev, spt_rev, 0.0,
                                         ALU.add, ALU.bypass)
            # texpx[l] = T[l] = exp(-incl[l+1]); col L-1 pre-set to 1.0
            nc.scalar.activation(texpx[:, 0:L - 1], incl[:, 1:L], AF.Exp,
                                 scale=-1.0)
            contrib = p2sb.tile([BL, L], f32)
            cap = p2sb.tile([BL, 1], f32)
            nc.vector.scalar_tensor_tensor(contrib[:], sp_all[0:32, :], 1.0,
                                           texpx[:], ALU.mult, ALU.mult,
                                           accum_out=cap[:])

            outc = p2sb.tile([BL, 1], f32)
            nc.vector.tensor_add(outc[:], dps, cap[:])
            nc.vector.tensor_scalar_add(outc[:], outc[:], scal_t[0:BL, 2:3])
            nc.sync.dma_start(out_d[:], outc[:])

    nc.compile()
    return nc


_CACHE = {}


def _prep_inputs(inputs):
    f = lambda a: np.ascontiguousarray(np.asarray(a, dtype=np.float32))
    Xg, Xl = f(inputs["Xg"]), f(inputs["Xl"])
    W1, b1 = f(inputs["W1"]), f(inputs["b1"])
    W2, b2 = f(inputs["W2"]), f(inputs["b2"])
    wa, ba = f(inputs["wa"]), f(inputs["ba"])
    wt, bt = f(inputs["wt"]), f(inputs["bt"])
    Wd1, bd1 = f(inputs["Wd1"]), f(inputs["bd1"])
    Wd2, bd2 = f(inputs["Wd2"]), f(inputs["bd2"])

    shared = {
        "w1s": np.ascontiguousarray(np.concatenate([W1, W1], axis=0)).astype(ml_dtypes.bfloat16),
        # [k, kc', ko, mc, m]: W2 row = kc'*256 + ko*128 + k, col = mc*128 + m
        "w2f8": np.ascontiguousarray(
            W2.reshape(2, 2, 128, 4, 128).transpose(2, 0, 1, 3, 4)
            .reshape(128, 2048)).astype(ml_dtypes.float8_e4m3),
        "wawt": np.ascontiguousarray(
            np.concatenate([wa, wt], axis=1).reshape(4, 128, 2)
            .transpose(1, 0, 2).reshape(128, 8)).astype(ml_dtypes.bfloat16),
        "wd1g": np.ascontiguousarray(Wd1[:G]),
        "wd1p": np.ascontiguousarray(
            (Wd1[G:] / np.float32(L)).reshape(4, 128, DH)
            .transpose(1, 0, 2).reshape(128, 4 * DH)),
        "wd2": np.ascontiguousarray(Wd2.reshape(2, 128).T),
        "b1": np.ascontiguousarray(b1.reshape(4, 128).T),
        "b2": np.ascontiguousarray(b2.reshape(4, 128).T),
        "bd1": np.ascontiguousarray(bd1.reshape(2, 128).T),
    }
    scal = np.zeros((128, 4), np.float32)
    scal[0:32, 0] = ba.reshape(-1)[0]
    scal[32:64, 0] = bt.reshape(-1)[0]
    scal[:, 2] = bd2.reshape(-1)[0]
    shared["scal"] = scal

    in_maps = []
    for c in range(NCORES):
        s = slice(c * BL, (c + 1) * BL)
        m = dict(shared)
        # [sl*64+f, g*1024 + h*512 + l]: 2KB-contiguous per partition row
        # per block so each block's fetch is 128 descriptors, not 256
        m["xlt"] = np.ascontiguousarray(
            Xl[s].reshape(NBLK, 2, 2, L, FD).transpose(2, 4, 0, 1, 3)
            .reshape(128, NBLK * 1024)).astype(ml_dtypes.bfloat16)
        m["xgt"] = np.ascontiguousarray(Xg[s].T)
        in_maps.append(m)
    return in_maps


def _run(inputs, trace=False, tmpdir=None):
    if "nc" not in _CACHE:
        _CACHE["nc"] = _build()
    nc = _CACHE["nc"]
    in_maps = _prep_inputs(inputs)
    res = run_bass_kernel_spmd(nc, in_maps, list(range(NCORES)),
                               trace=trace, tmpdir=tmpdir)
    out = np.concatenate([res.results[c]["out"] for c in range(NCORES)], axis=0)
    return out.astype(np.float32), res


def kernel(**inputs) -> np.ndarray:
    out, _ = _run(inputs)
    return out


# revision 23
# speedup vs baseline: 1.0274x; 1.0274x over previous
"""Trainium2 Bass kernel for nn_CumulativeShadeRegressor.

Model (per sample): per-leaf MLP encoder [L, FD] -> [L, H2] (two gelu
layers), softplus absorb/atten heads, a top-to-bottom exponential
transmittance scan over L, mean-pooling over L, and a small dense head on
[Xg | pooled].

Strategy: data-parallel over B across 8 NeuronCores (32 samples/core).
Both the PE and the ACT engine are near-saturated in this problem, so the
kernel attacks both:
  * layer 1 (K=64) runs as row-tiled bf16 matmul pairs (2 concurrent MMs
    in disjoint 64-row PE strips);
  * layer 2 runs in fp8e4 DoubleRow mode (2 MACs/cell, contraction 256
    per pass) with h1 quantized to fp8 by the gelu ACT itself;
  * gelu ACT ops are batched to N=2048 (4 samples per instruction, PSUM
    pair ring of 2x4 banks) to amortize the ~222-cycle ACT init;
  * per-sample pooling is a DVE 3D-AP reduce (off the ACT critical path);
  * absorb/atten head matmuls (M=2) run post-loop as col-tiled bursts
    (4 samples concurrently in 32-col PE strips);
  * the tail uses softplus(x)=ln(1+exp(x)) and T=exp(-cumsum), all inside
    the single natural_log_exp ACT table set (one table switch total).
"""
import sys

sys.path.insert(0, "/opt/trn_rl_repo")

import numpy as np
import ml_dtypes

import concourse.bacc as bacc
import concourse.mybir as mybir
import concourse.tile as tile
from concourse.bass_utils import run_bass_kernel_spmd
from concourse.tile import add_dep_helper

B, L, FD, G = 256, 512, 64, 32
H1, H2, DH = 512, 512, 256
NCORES = 8
BL = B // NCORES          # 32 samples per core
NBLK = BL // 4            # 8 blocks of 4 samples

f32 = mybir.dt.float32
bf16 = mybir.dt.bfloat16
f8e4 = mybir.dt.float8e4
AF = mybir.ActivationFunctionType
ALU = mybir.AluOpType
AX = mybir.AxisListType
DR = mybir.MatmulPerfMode.DoubleRow


def _build():
    nc = bacc.Bacc("TRN2", target_bir_lowering=False, debug=False,
                   num_devices=NCORES)

    d = {}
    d["xlt"] = nc.dram_tensor("xlt", [128, NBLK * 1024], bf16, kind="ExternalInput").ap()
    d["xgt"] = nc.dram_tensor("xgt", [G, BL], f32, kind="ExternalInput").ap()
    d["w1s"] = nc.dram_tensor("w1s", [128, H1], bf16, kind="ExternalInput").ap()
    d["w2f8"] = nc.dram_tensor("w2f8", [128, 2048], f8e4, kind="ExternalInput").ap()
    d["wawt"] = nc.dram_tensor("wawt", [128, 8], bf16, kind="ExternalInput").ap()
    d["wd1g"] = nc.dram_tensor("wd1g", [G, DH], f32, kind="ExternalInput").ap()
    d["wd1p"] = nc.dram_tensor("wd1p", [128, 4 * DH], f32, kind="ExternalInput").ap()
    d["wd2"] = nc.dram_tensor("wd2", [128, 2], f32, kind="ExternalInput").ap()
    d["b1"] = nc.dram_tensor("b1", [128, 4], f32, kind="ExternalInput").ap()
    d["b2"] = nc.dram_tensor("b2", [128, 4], f32, kind="ExternalInput").ap()
    d["bd1"] = nc.dram_tensor("bd1", [128, 2], f32, kind="ExternalInput").ap()
    d["scal"] = nc.dram_tensor("scal", [128, 4], f32, kind="ExternalInput").ap()
    out_d = nc.dram_tensor("out", [BL, 1], f32, kind="ExternalOutput").ap()

    with tile.TileContext(nc) as tc:
        with (
            tc.tile_pool(name="wp", bufs=1) as wp,
            tc.tile_pool(name="pp", bufs=1) as pp,
            tc.tile_pool(name="xp", bufs=4) as xp,
            tc.tile_pool(name="h1p", bufs=2) as h1p,
            tc.tile_pool(name="p2sb", bufs=1) as p2sb,
            tc.tile_pool(name="awp", bufs=2) as awp,
            tc.tile_pool(name="psp", bufs=1, space="PSUM") as psp,
        ):
            w1s_t = wp.tile([128, H1], bf16)
            w2f8_t = wp.tile([128, 2048], f8e4)
            wawt_t = wp.tile([128, 8], bf16)
            xgt_t = wp.tile([G, BL], f32)
            wd1g_t = wp.tile([G, DH], f32)
            wd1p_t = wp.tile([128, 4 * DH], f32)
            wd2_t = wp.tile([128, 2], f32)
            b1_t = wp.tile([128, 4], f32)
            b2_t = wp.tile([128, 4], f32)
            bd1_t = wp.tile([128, 2], f32)
            scal_t = wp.tile([128, 4], f32)

            # scratch + dummy gelu first: the ~1.3us gelu ACT_TABLE_LOAD
            # runs concurrently with the input DMAs
            wu_sb = wp.tile([128, 128], f32, name="wu_sb")
            scr_t = wp.tile([1, 4], f32, name="scr_t")
            texpx = p2sb.tile([BL, L], f32, name="texpx")
            nc.gpsimd.memset(wu_sb[:], 0.0)
            nc.gpsimd.memset(texpx[:, L - 1:L], 1.0)
            nc.scalar.activation(scr_t[0:1, 0:1], wu_sb[0:1, 0:1], AF.Gelu)

            # first block's inputs land before everything else
            x2_pre = {}

            def fetch_x2(g):
                xt = xp.tile([128, 2 * L], bf16, name=f"x2_{g}", tag="x2")
                nc.sync.dma_start(xt[:], d["xlt"][:, g * 1024:(g + 1) * 1024])
                x2_pre[g] = xt

            fetch_x2(0)
            nc.sync.dma_start(w1s_t[:], d["w1s"][:])
            nc.sync.dma_start(b1_t[:], d["b1"][:])
            fetch_x2(1)
            fetch_x2(2)
            for nm, t in [("w2f8", w2f8_t), ("b2", b2_t), ("wawt", wawt_t)]:
                nc.sync.dma_start(t[:], d[nm][:])
            for nm, t in [("xgt", xgt_t), ("wd1g", wd1g_t), ("wd1p", wd1p_t),
                          ("wd2", wd2_t), ("bd1", bd1_t), ("scal", scal_t)]:
                nc.gpsimd.dma_start(t[:], d[nm][:])

            pooled_t = pp.tile([128, 4 * BL], f32)   # [h_part, mc*32 + s]
            h2all = pp.tile([128, NBLK * 4 * 2048], bf16)  # [feat, (g*4+mc)*2048 + j*512 + l]

            # all of PSUM as one tile; pairs P0=[0:2048], P1=[2048:4096]
            ps_all = psp.tile([128, 4096], f32)

            # PE warm-up: back-to-back matmuls on scratch data so the HAM
            # clock gate reaches K=8/8 before (and until) the real work.
            for i in range(10):
                nc.tensor.matmul(ps_all[:, 3968:4096], wu_sb[:], wu_sb[:],
                                 start=True, stop=True)

            h1tiles = {}
            unit = 0  # ACT-unit counter; parity picks the PSUM pair

            def l1_unit(g, mc):
                """One layer-1 ACT unit: 4 row-tiled bf16 MM pairs + gelu->fp8."""
                nonlocal unit
                x2t = x2_pre[g]
                h1t = h1tiles[g]
                pbase = (unit % 2) * 2048
                for h in range(2):
                    for sl in range(2):
                        nc.tensor.matmul(
                            ps_all[:, pbase + (2 * h + sl) * 512:
                                   pbase + (2 * h + sl) * 512 + 512],
                            w1s_t[64 * sl:64 * sl + 64, mc * 128:(mc + 1) * 128],
                            x2t[64 * sl:64 * sl + 64, h * 512:(h + 1) * 512],
                            start=True, stop=True)
                nc.scalar.activation(
                    h1t[:, mc * 2048:(mc + 1) * 2048],
                    ps_all[:, pbase:pbase + 2048],
                    AF.Gelu, bias=b1_t[:, mc:mc + 1])
                unit += 1

            def l2_unit(g, mc):
                """One layer-2 ACT unit: 8 fp8 DoubleRow MMs + gelu + pooling."""
                nonlocal unit
                h1t = h1tiles[g]
                pbase = (unit % 2) * 2048
                for kcp in range(2):
                    wk = w2f8_t[:, kcp * 1024:(kcp + 1) * 1024].rearrange(
                        "p (ko mcm) -> p ko mcm", ko=2)
                    hk = h1t[:, (2 * kcp) * 2048:(2 * kcp + 2) * 2048].rearrange(
                        "p (ko n) -> p ko n", ko=2)
                    for j in range(4):
                        nc.tensor.matmul(
                            ps_all[:, pbase + j * 512:pbase + (j + 1) * 512],
                            wk[:, :, mc * 128:(mc + 1) * 128],
                            hk[:, :, j * 512:(j + 1) * 512],
                            start=(kcp == 0), stop=(kcp == 1),
                            perf_mode=DR)
                h2base = (g * 4 + mc) * 2048
                nc.scalar.activation(
                    h2all[:, h2base:h2base + 2048],
                    ps_all[:, pbase:pbase + 2048],
                    AF.Gelu, bias=b2_t[:, mc:mc + 1])
                unit += 1
                # per-sample pooling: sum over L on the DVE
                nc.vector.reduce_sum(
                    pooled_t[:, mc * BL + g * 4:mc * BL + g * 4 + 4],
                    h2all[:, h2base:h2base + 2048].rearrange(
                        "p (j n) -> p j n", j=4),
                    axis=AX.X)

            # prologue: block 0's layer 1, with fillers to bridge the first
            # ACT-latency gap so HAM stays at K=8/8
            h1tiles[0] = h1p.tile([128, 4 * 2048], f8e4, name="h1t_0", tag="h1t")
            for mc in range(4):
                l1_unit(0, mc)
                if mc == 0:
                    for i in range(12):
                        nc.tensor.matmul(ps_all[:, 3968:4096], wu_sb[:],
                                         wu_sb[:], start=True, stop=True)

            # main loop, software-pipelined: block g+1's layer-1 units
            # interleave with block g's layer-2 units so the PE never idles
            # long enough for HAM to re-throttle.
            for g in range(NBLK):
                if g + 3 < NBLK:
                    fetch_x2(g + 3)
                if g + 1 < NBLK:
                    h1tiles[g + 1] = h1p.tile([128, 4 * 2048], f8e4,
                                              name=f"h1t_{g+1}", tag="h1t")
                for mc in range(4):
                    if g + 1 < NBLK:
                        l1_unit(g + 1, mc)
                    l2_unit(g, mc)

            # ---- phase 2 ----
            # absorb/atten pre-acts first (they gate the serial tail; the
            # dense head would head-of-line-block the PE FIFO waiting on
            # pooled_t): col-tiled bursts, 4 samples concurrent.
            # psum bank alternates between [0:512] and [512:1024].
            aw_all = p2sb.tile([64, L], f32)  # rows 0-31 absorb, 32-63 atten
            d1t = []
            gelu_insts = []

            def absorb_burst(g):
                abank = (g % 2) * 512
                for c in range(4):
                    for j in range(4):
                        nc.tensor.matmul(
                            ps_all[32 * j:32 * j + 2, abank:abank + 512],
                            wawt_t[:, 2 * c:2 * c + 2],
                            h2all[:, (g * 4 + c) * 2048 + j * 512:
                                  (g * 4 + c) * 2048 + (j + 1) * 512],
                            start=(c == 0), stop=(c == 3),
                            tile_position=(0, 32 * j))
                aw_sb = awp.tile([128, L], f32, name=f"aw_sb_{g}", tag="aw_sb")
                # scalar engine (idle here) does the drain; the DVE is still
                # busy with the last block's pooling reduces
                nc.scalar.copy(aw_sb[:], ps_all[:, abank:abank + 512])
                nc.sync.dma_start(aw_all[g * 4:g * 4 + 4, :], aw_sb[0:128:32, :])
                nc.sync.dma_start(aw_all[32 + g * 4:32 + g * 4 + 4, :],
                                  aw_sb[1:128:32, :])

            def dense_head_in(mc2):
                dbase = 1024 + mc2 * 512
                ps = ps_all[:, dbase:dbase + BL]
                nc.tensor.matmul(ps, wd1g_t[:, mc2 * 128:(mc2 + 1) * 128],
                                 xgt_t[:], start=True, stop=False)
                for hc in range(4):
                    nc.tensor.matmul(
                        ps,
                        wd1p_t[:, hc * DH + mc2 * 128:hc * DH + (mc2 + 1) * 128],
                        pooled_t[:, hc * BL:(hc + 1) * BL],
                        start=False, stop=(hc == 3))
                t = p2sb.tile([128, BL], f32, name=f"d1t_{mc2}")
                gi = nc.scalar.activation(t[:], ps, AF.Gelu,
                                          bias=bd1_t[:, mc2:mc2 + 1])
                gelu_insts.append(gi)
                d1t.append(t)

            # bursts 0-3 run while pooled_t finishes; the dense head slots in
            # between so its gelus (and the tail's table load) overlap the
            # remaining bursts instead of waiting behind all 8.
            for g in range(4):
                absorb_burst(g)
            dense_head_in(0)
            dense_head_in(1)
            # load the exp/ln table set now: it's the last table switch, and
            # placing it here lets it hide under the remaining bursts
            from concourse.hw_specs import get_activation_tables
            tabs = get_activation_tables(nc.m.arch)
            set_id = next(i for i, fns in enumerate(tabs.values())
                          if AF.Exp in fns and AF.Ln in fns)
            li = nc.scalar.add_instruction(mybir.InstLoadActFuncSet(
                name=nc.scalar.bass.get_next_instruction_name(),
                act_func_set_id=set_id, ins=[], outs=[]))
            for gi in gelu_insts:
                add_dep_helper(li.ins, gi.ins, sync=True,
                               reason="ACT table set order: gelu before ln/exp")
            absorb_burst(4)
            absorb_burst(5)
            dps = ps_all[0:BL, 2048:2049]
            nc.tensor.matmul(dps, d1t[0][:], wd2_t[:, 0:1], start=True, stop=False)
            nc.tensor.matmul(dps, d1t[1][:], wd2_t[:, 1:2], start=False, stop=True)
            absorb_burst(6)
            absorb_burst(7)

            # tail: softplus via ln(1+exp), T via exp(-cumsum); one table set
            # (loaded above, between the head gelus and the late bursts).
            # scal col0 rows 0-31 = ba, rows 32-63 = bt.
            e_aw = p2sb.tile([64, L], f32)
            ei = nc.scalar.activation(e_aw[:], aw_all[:], AF.Exp,
                                      bias=scal_t[0:64, 0:1])
            add_dep_helper(ei.ins, li.ins, sync=True,
                           reason="ACT table set order: ln set before exp")
            # softplus = ln(1 + e): the +1 rides the ACT bias for free
            sp_all = p2sb.tile([64, L], f32)
            nc.scalar.activation(sp_all[:], e_aw[:], AF.Ln, bias=1.0)
            # incl[l] = sum_{l'>=l} softplus(atten): add-scan over reversed L
            incl = p2sb.tile([BL, L], f32)
            spt_rev = sp_all[32:64, L - 1::-1]
            incl_rev = incl[:, L - 1::-1]
            nc.vector.tensor_tensor_scan(incl_rev, spt_rev, spt_rev, 0.0,
                                         ALU.add, ALU.bypass)
            # texpx[l] = T[l] = exp(-incl[l+1]); col L-1 pre-set to 1.0
            nc.scalar.activation(texpx[:, 0:L - 1], incl[:, 1:L], AF.Exp,
                                 scale=-1.0)
            contrib = p2sb.tile([BL, L], f32)
            cap = p2sb.tile([BL, 1], f32)
            nc.vector.scalar_tensor_tensor(contrib[:], sp_all[0:32, :], 1.0,
                                           texpx[:], ALU.mult, ALU.mult,
                                           accum_out=cap[:])

            outc = p2sb.tile([BL, 1], f32)
            nc.vector.tensor_add(outc[:], dps, cap[:])
            nc.vector.tensor_scalar_add(outc[:], outc[:], scal_t[0:BL, 2:3])
            nc.sync.dma_start(out_d[:], outc[:])

    nc.compile()
    return nc


_CACHE = {}


def _prep_inputs(inputs):
    f = lambda a: np.ascontiguousarray(np.asarray(a, dtype=np.float32))
    Xg, Xl = f(inputs["Xg"]), f(inputs["Xl"])
    W1, b1 = f(inputs["W1"]), f(inputs["b1"])
    W2, b2 = f(inputs["W2"]), f(inputs["b2"])
    wa, ba = f(inputs["wa"]), f(inputs["ba"])
    wt, bt = f(inputs["wt"]), f(inputs["bt"])
    Wd1, bd1 = f(inputs["Wd1"]), f(inputs["bd1"])
    Wd2, bd2 = f(inputs["Wd2"]), f(inputs["bd2"])

    shared = {
        "w1s": np.ascontiguousarray(np.concatenate([W1, W1], axis=0)).astype(ml_dtypes.bfloat16),
        # [k, kc', ko, mc, m]: W2 row = kc'*256 + ko*128 + k, col = mc*128 + m
        "w2f8": np.ascontiguousarray(
            W2.reshape(2, 2, 128, 4, 128).transpose(2, 0, 1, 3, 4)
            .reshape(128, 2048)).astype(ml_dtypes.float8_e4m3),
        "wawt": np.ascontiguousarray(
            np.concatenate([wa, wt], axis=1).reshape(4, 128, 2)
            .transpose(1, 0, 2).reshape(128, 8)).astype(ml_dtypes.bfloat16),
        "wd1g": np.ascontiguousarray(Wd1[:G]),
        "wd1p": np.ascontiguousarray(
            (Wd1[G:] / np.float32(L)).reshape(4, 128, DH)
            .transpose(1, 0, 2).reshape(128, 4 * DH)),
        "wd2": np.ascontiguousarray(Wd2.reshape(2, 128).T),
        "b1": np.ascontiguousarray(b1.reshape(4, 128).T),
        "b2": np.ascontiguousarray(b2.reshape(4, 128).T),
        "bd1": np.ascontiguousarray(bd1.reshape(2, 128).T),
    }
    scal = np.zeros((128, 4), np.float32)
    scal[0:32, 0] = ba.reshape(-1)[0]
    scal[32:64, 0] = bt.reshape(-1)[0]
    scal[:, 2] = bd2.reshape(-1)[0]
    shared["scal"] = scal

    in_maps = []
    for c in range(NCORES):
        s = slice(c * BL, (c + 1) * BL)
        m = dict(shared)
        # [sl*64+f, g*1024 + h*512 + l]: 2KB-contiguous per partition row
        # per block so each block's fetch is 128 descriptors, not 256
        m["xlt"] = np.ascontiguousarray(
            Xl[s].reshape(NBLK, 2, 2, L, FD).transpose(2, 4, 0, 1, 3)
            .reshape(128, NBLK * 1024)).astype(ml_dtypes.bfloat16)
        m["xgt"] = np.ascontiguousarray(Xg[s].T)
        in_maps.append(m)
    return in_maps


def _run(inputs, trace=False, tmpdir=None):
    if "nc" not in _CACHE:
        _CACHE["nc"] = _build()
    nc = _CACHE["nc"]
    in_maps = _prep_inputs(inputs)
    res = run_bass_kernel_spmd(nc, in_maps, list(range(NCORES)),
                               trace=trace, tmpdir=tmpdir)
    out = np.concatenate([res.results[c]["out"] for c in range(NCORES)], axis=0)
    return out.astype(np.float32), res


def kernel(**inputs) -> np.ndarray:
    out, _ = _run(inputs)
    return out
